# revision 17
# baseline (speedup 1.0000x reference)
"""Transformer encoder layer (Informer-style) Bass/Tile kernel for TRN2.

394753 ns per core (TimelineSim), rel err 1.71e-2; from the 458916 ns bf16
baseline. Data-parallel over batch: one [S=1024, D=1024] layer per core x8.

What this version adds on top of the bf16 kernel (fp8 e4m3 DoubleRow):
 - qkv / out-proj / fc1 matmuls run in fp8 DoubleRow perf mode: lhsT
   [K, 2, M<=64], rhs [K, 2, N<=256], 0.5 cycles/row -- 2x PE throughput
   with 2x contraction packed per instruction (d-tile pairs 2t, 2t+1).
 - weights are scaled x64 into fp8 (std 0.02 -> 1.28, clear of e4m3
   subnormals); every descale folds into an op that already exists:
     exp(scale=2^-15) eats SCALING/64^2 for the scores,
     gelu(scale=2^-6) eats fc1's x64,
     the v_aug ones-column is 8.0 so the softmax division emits
       attnC = 8*attn straight into fp8,
     the hs residual is pre-scaled x512 on the host (LN1 is
       scale-invariant, so 512*(hs + attn@wo) normalizes identically).
 - scores stay bf16 (contract=64 gains nothing from DoubleRow: cost is
   moving-dim-bound either way); fc2 stays bf16 (fc2-fp8 pushes rel err
   to ~2.6e-2, past the 2e-2 gate; fc1-fp8 alone measures 1.71e-2).
 - fc2(q-1) chains interleave into fc1(q)'s stretch: gelu evictions
   (1.03us per [64, S] on ACT -- half-lane ops, M<=64) otherwise
   rate-limit the 0.85us fc1 chains via the 3-buf psum rotation; FFN
   weight quarters are double-buffered and prefetched one quarter ahead.
 - residual adds read the DR psum halves [64, D] directly with the SB
   operand at a shifted partition base (legal: the equal-base rule only
   binds when both inputs are in SB).
 - x1T for fc1 = bf16 DMA-XBAR transpose (1-byte transpose unsupported)
   + per-ts Pool/DVE converts to a packed fp8 x1T8.
 - LN2 residual is pre-merged on Pool (out2 += x1) and added after the
   last fc2 chain on DVE: a DVE preload written INTO psum is silently
   wrong on banks whose previous matmul group ran fp8-DR (found the hard
   way; CoreSim models it correctly, hardware does not).

HW-verified fp8-DR semantics (dr_test*.py): interp pairing matches HW;
matmul psum dst must start at partition 0 (s3d3 ISA check) -- M-halves go
to separate psum tiles, offset-shifted engine writes merge them; GPSIMD
cannot touch PSUM; ACT scale/bias APs are read at input lanes, so
offset-shifted activation evictions work.

SBUF plan (KB/partition): const .6 | w2 16x2 | w1(=wv slot) 8x2 | hsT8 8
| hs/out2 32 | x1b16 16 + x1T 2x8 + x1T8 8 | attnC 8 | wo8 8 | v_aug
16.25 | qk 6 | wqk 2x2 | probs 10 | hT 2x16 | tmp ~4.
"""

from contextlib import ExitStack

import concourse.bass as bass
import concourse.mybir as mybir
import concourse.tile as tile
from concourse import bacc

AFT = mybir.ActivationFunctionType
ALU = mybir.AluOpType
F32 = mybir.dt.float32
F32R = mybir.dt.float32r
BF16 = mybir.dt.bfloat16

P = 128
S = 1024
D = 1024
H = 16
HD = 64
F = 4096
NTS = S // P   # 8
NTD = D // P   # 8
FQ = 1024      # fc1/fc2 f-quarter size
NQ = F // FQ   # 4
FQT = FQ // P  # 8
EPS = 1e-5
NCH = 2
CW = 512


def build(fast=True):
    nc = bacc.Bacc("TRN2", target_bir_lowering=False, debug=False)

    def din(name, shape, dt=F32):
        return nc.dram_tensor(name, shape, dt, kind="ExternalInput").ap()

    io = dict(
        hsT=din("hsT", (D, S), BF16),
        hs=din("hs", (S, D)),
        # wq/wk in hp-blocked, per-partition-contiguous layout:
        # wqb[hp][p, ti*P + c] = (wq.T * SCALING)[ti*P + p, hp*P + c].
        # 2048-byte descriptor runs instead of 256-byte (which pay a 2x
        # small-element DMA penalty).
        wqb=din("wqb", (H // 2, P, D), BF16),
        wkb=din("wkb", (H // 2, P, D), BF16),
        wvT=din("wvT", (D, D), BF16),
        woT=din("woT", (D, D), BF16),
        bq=din("bq", (D,)),             # * SCALING
        bk=din("bk", (D,)),
        bv=din("bv", (D,)),
        bo=din("bo", (D,)),
        g1=din("g1", (D,)),
        b1=din("b1", (D,)),
        g2=din("g2", (D,)),
        b2=din("b2", (D,)),
        f1w=din("f1w", (D, F), BF16),   # fc1_w.T
        f1b=din("f1b", (F,)),
        f2w=din("f2w", (F, D), BF16),   # fc2_w.T
        f2b=din("f2b", (D,)),
        out=nc.dram_tensor("out", (S, D), F32, kind="ExternalOutput").ap(),
        fast=fast,
    )

    with tile.TileContext(nc) as tc:
        _body(tc, io)
    nc.compile()
    return nc


def _body(tc, t):
    nc = tc.nc
    fast = t["fast"]
    hsT, hs = t["hsT"], t["hs"]
    wqb, wkb, wvT, woT = t["wqb"], t["wkb"], t["wvT"], t["woT"]
    bq, bk, bv, bo = t["bq"], t["bk"], t["bv"], t["bo"]
    g1, b1, g2, b2 = t["g1"], t["b1"], t["g2"], t["b2"]
    f1w, f1b, f2w, f2b = t["f1w"], t["f1b"], t["f2w"], t["f2b"]
    out = t["out"]

    # ---- pools, in LIFO release order (bottom of stack first) ----
    const = tc.alloc_tile_pool(name="const", bufs=1)
    f2wp = tc.alloc_tile_pool(name="f2wp", bufs=2)
    f1wp = tc.alloc_tile_pool(name="f1wp", bufs=2)
    big16 = tc.alloc_tile_pool(name="big16", bufs=2)
    big32 = tc.alloc_tile_pool(name="big32", bufs=1)
    x1bp = tc.alloc_tile_pool(name="x1bp", bufs=1)
    psU = tc.alloc_tile_pool(name="psU", bufs=3, space="PSUM")
    psQ = tc.alloc_tile_pool(name="psQ", bufs=2, space="PSUM")
    wo_pool = tc.alloc_tile_pool(name="wo_pool", bufs=1)
    vaug_pool = tc.alloc_tile_pool(name="vaug_pool", bufs=1)

    eps_t = const.tile([P, 1], F32, tag="eps")
    nc.vector.memset(eps_t, EPS)
    ones_t = const.tile([P, 1], F32, tag="ones")
    nc.vector.memset(ones_t, 1.0)
    if not fast:
        bqk_t = const.tile([P, 2, NTD], F32, tag="bqk")
        nc.sync.dma_start(out=bqk_t[:, 0, :], in_=bq.rearrange("(t p) -> p t", p=P))
        nc.sync.dma_start(out=bqk_t[:, 1, :], in_=bk.rearrange("(t p) -> p t", p=P))
        f1b_t = const.tile([P, F // P], F32, tag="f1b")
        nc.sync.dma_start(out=f1b_t, in_=f1b.rearrange("(t p) -> p t", p=P))

    def bcast_tile(pool, src, queue=None):
        bt = pool.tile([P, D], F32, tag="bc" + src.name, name="bc" + src.name)
        (queue or nc.sync).dma_start(
            out=bt, in_=src.unsqueeze(0).broadcast_to((P, D)))
        return bt

    # ---- persistent tiles ----
    hsT_sb = big16.tile([P, NTD, S], BF16, tag="big", name="hsT_sb")
    hs_sb = big32.tile([P, NTS, D], F32, tag="big", name="hs_sb")
    v_aug = vaug_pool.tile([P, NTS, H, HD + 1], BF16, tag="vaug")
    wo_sb = wo_pool.tile([P, NTD, D], BF16, tag="wo")
    wv_sb = f2wp.tile([P, NTD, D], BF16, tag="w2", name="wv_sb")

    # ---- initial loads (batched: one strided dma_start per tensor — the
    # ~1.3us per-DMA issue overhead on the SEQ/HWDGE otherwise limits the
    # startup feed rate far below DMA bandwidth) ----
    hsT_r = hsT.rearrange("(t p) s -> p t s", p=P)
    wvT_r = wvT.rearrange("(t p) d -> p t d", p=P)
    hf = NTD // 2
    if not fast:
        bv_bc = bcast_tile(vaug_pool, bv, queue=nc.sync)

    f1w_r = f1w.rearrange("(t p) f -> p t f", p=P)
    f2w_r = f2w.rearrange("(t p) d -> p t d", p=P)

    def load_quarter(q):
        w1 = f1wp.tile([P, NTD, FQ], BF16, tag="w1", name=f"w1q{q}")
        nc.sync.dma_start(out=w1, in_=f1w_r[:, :, q * FQ:(q + 1) * FQ])
        w2 = f2wp.tile([P, FQT, D], BF16, tag="w2", name=f"w2q{q}")
        nc.sync.dma_start(out=w2, in_=f2w_r[:, q * FQT:(q + 1) * FQT, :])
        return w1, w2

    # ones column of v_aug (denominator trick)
    nc.vector.tensor_copy(
        out=v_aug[:, :, :, HD:HD + 1],
        in_=ones_t.unsqueeze(1).unsqueeze(1).broadcast_to((P, NTS, H, 1)))

    # ---------------- fused attention pools (alloc before qk0) ----------------
    fused = ExitStack()
    qk_pool = fused.enter_context(tc.tile_pool(name="qkt", bufs=3))
    wqk_pool = fused.enter_context(tc.tile_pool(name="wqkp", bufs=2))
    probs_pool = fused.enter_context(tc.tile_pool(name="probs", bufs=5))
    bc_pool = fused.enter_context(tc.tile_pool(name="bcp", bufs=3))
    rr_pool = fused.enter_context(tc.tile_pool(name="rrp", bufs=4))
    stg_pool = fused.enter_context(tc.tile_pool(name="stgp", bufs=2))

    def qk_issue(hp):
        st = {"hp": hp}
        for wsrc, nm in ((wqb, "q"), (wkb, "k")):
            wblk = wqk_pool.tile([P, NTD, P], BF16, tag="w" + nm, name="w" + nm)
            nc.sync.dma_start(out=wblk, in_=wsrc[hp])
            st["w" + nm] = wblk
            st[nm] = qk_pool.tile([P, S], BF16, tag=nm + "T", name=nm + "Th")
        return st

    def qk_copy(st, nm, nch, ps):
        if fast:
            nc.vector.tensor_copy(
                out=st[nm][:, nch * CW:(nch + 1) * CW], in_=ps)
        else:
            bidx = 0 if nm == "q" else 1
            nc.vector.tensor_scalar_add(
                out=st[nm][:, nch * CW:(nch + 1) * CW], in0=ps,
                scalar1=bqk_t[:, bidx, st["hp"]:st["hp"] + 1])

    def qk_compute(st, interleave=False):
        # interleave=True (head-pair 0 only): the two nch chains advance in
        # two-step pieces so each freshly-landed hsT quarter feeds ~850ns of
        # PE work instead of ~430ns — halves the DMA-feed stalls at startup.
        for nm in ("q", "k"):
            if interleave:
                pss = [psQ.tile([P, CW], F32, tag="q5", name="psq")
                       for _ in range(NCH)]
                for ti0 in range(0, NTD, 2):
                    for nch in range(NCH):
                        for ti in (ti0, ti0 + 1):
                            nc.tensor.matmul(
                                pss[nch], lhsT=st["w" + nm][:, ti, :],
                                rhs=hsT_sb[:, ti, nch * CW:(nch + 1) * CW],
                                start=(ti == 0), stop=(ti == NTD - 1))
                for nch in range(NCH):
                    qk_copy(st, nm, nch, pss[nch])
            else:
                for nch in range(NCH):
                    ps = psQ.tile([P, CW], F32, tag="q5", name="psq")
                    for ti in range(NTD):
                        nc.tensor.matmul(
                            ps, lhsT=st["w" + nm][:, ti, :],
                            rhs=hsT_sb[:, ti, nch * CW:(nch + 1) * CW],
                            start=(ti == 0), stop=(ti == NTD - 1))
                    qk_copy(st, nm, nch, ps)
        return st

    def qk_pieces(st):
        """The qk projection of head-pair hp+2 sliced into 16 two-matmul
        thunks, interleaved one per tk iteration of the current heads: the
        PE then always has slack work while ACT streams the exps, instead
        of a solid qk block during which ACT drains and then starves."""
        thunks = []
        for nm in ("q", "k"):
            for nch in range(NCH):
                holder = {}
                for ti0 in range(0, NTD, 2):
                    def piece(nm=nm, nch=nch, ti0=ti0, holder=holder, st=st):
                        if ti0 == 0:
                            holder["ps"] = psQ.tile([P, CW], F32, tag="q5",
                                                    name="psq")
                        ps = holder["ps"]
                        for ti in (ti0, ti0 + 1):
                            nc.tensor.matmul(
                                ps, lhsT=st["w" + nm][:, ti, :],
                                rhs=hsT_sb[:, ti, nch * CW:(nch + 1) * CW],
                                start=(ti == 0), stop=(ti == NTD - 1))
                        if ti0 == NTD - 2:
                            qk_copy(st, nm, nch, ps)
                    thunks.append(piece)
        return thunks

    # qk of head-pairs 0 and 1 first: they only need hsT + small weight
    # blocks. The startup loads interleave so qk0's chain can begin as soon
    # as wq0 + the first hsT quarter land and then stream; wv follows (the
    # v-proj only starts after ~14us of qk0/qk1 PE work).
    st0 = {"hp": 0}
    st0["wq"] = wqk_pool.tile([P, NTD, P], BF16, tag="wq", name="wq")
    nc.sync.dma_start(out=st0["wq"], in_=wqb[0])
    for i in range(0, 2):
        nc.sync.dma_start(out=hsT_sb[:, i, :], in_=hsT_r[:, i, :])
    st0["wk"] = wqk_pool.tile([P, NTD, P], BF16, tag="wk", name="wk")
    nc.sync.dma_start(out=st0["wk"], in_=wkb[0])
    for i in range(2, NTD):
        nc.sync.dma_start(out=hsT_sb[:, i, :], in_=hsT_r[:, i, :])
    st0["q"] = qk_pool.tile([P, S], BF16, tag="qT", name="qTh")
    st0["k"] = qk_pool.tile([P, S], BF16, tag="kT", name="kTh")
    qk_pipe = [qk_compute(st0, interleave=True), qk_compute(qk_issue(1))]

    qt = NTD // 4
    for i in range(4):
        nc.sync.dma_start(out=wv_sb[:, i * qt:(i + 1) * qt, :],
                          in_=wvT_r[:, i * qt:(i + 1) * qt, :])

    # ---------------- v projection ----------------
    # hsT stationary, wvT moving -> token-major v_aug (bf16)
    for ts in range(NTS):
        ps = psU.tile([P, D], F32, tag="u", name="psv")
        for nch in range(NCH):
            for ti in range(NTD):
                nc.tensor.matmul(
                    ps[:, nch * CW:(nch + 1) * CW],
                    lhsT=hsT_sb[:, ti, ts * P:(ts + 1) * P],
                    rhs=wv_sb[:, ti, nch * CW:(nch + 1) * CW],
                    start=(ti == 0), stop=(ti == NTD - 1))
        if fast:
            nc.vector.tensor_copy(
                out=v_aug[:, ts, :, 0:HD],
                in_=ps.rearrange("p (h e) -> p h e", h=H))
        else:
            nc.vector.tensor_tensor(
                out=v_aug[:, ts, :, 0:HD],
                in0=ps.rearrange("p (h e) -> p h e", h=H),
                in1=bv_bc.rearrange("p (h e) -> p h e", h=H),
                op=ALU.add)

    # ---------------- attention (qk prefetched two head-pairs ahead) -------
    attnC = big16.tile([P, NTD, S], BF16, tag="big", name="attnC")
    hs_r = hs.rearrange("(t p) d -> p t d", p=P)
    ffn_w0 = None

    piece_q = []
    for hp in range(H // 2):
        qkh = qk_pipe.pop(0)
        if hp + 2 < H // 2:
            st_next = qk_issue(hp + 2)
            qk_pipe.append(st_next)
            piece_q.extend(qk_pieces(st_next))
        if hp == 0:
            # residual hs, issued now so it doesn't delay the wqk blocks
            # above it on the scalar queue (needed only from out-proj on)
            nc.sync.dma_start(out=hs_sb, in_=hs_r)
            if not fast:
                bo_bc = bcast_tile(wo_pool, bo, queue=nc.sync)
                for ts in range(NTS):
                    nc.vector.tensor_tensor(out=hs_sb[:, ts, :],
                                            in0=hs_sb[:, ts, :],
                                            in1=bo_bc, op=ALU.add)
        if hp == 1:
            nc.sync.dma_start(
                out=wo_sb, in_=woT.rearrange("(t p) d -> p t d", p=P))
            # FFN quarter-0 prefetch; the wv slot is recycled for w2 quarters
            ffn_w0 = load_quarter(0)
        for h in (2 * hp, 2 * hp + 1):
            r0 = (h % 2) * HD
            last_hp = (hp == H // 2 - 1)
            if last_hp:
                ps_at = [psQ.tile([P, CW], F32, tag="q5", name=f"atq{h}{c}")
                         for c in range(NCH)]
            else:
                at_full = psU.tile([P, S], F32, tag="u", name=f"at{h}")
                ps_at = [at_full[:, c * CW:(c + 1) * CW] for c in range(NCH)]
            def at_matmuls(tk, pr):
                for nch in range(NCH):
                    nc.tensor.matmul(
                        ps_at[nch][0:HD + 1, :],
                        lhsT=v_aug[:, tk, h, :],
                        rhs=pr[:, nch * CW:(nch + 1) * CW],
                        start=(tk == 0), stop=(tk == NTS - 1))

            # software-pipelined two deep: at(tk-2) runs behind sc(tk). The
            # exp->attn slack at one-deep (~126ns) almost exactly equals the
            # semaphore propagation cost (117ns), so every iteration paid
            # one sem-prop; two-deep makes the slack a full iteration.
            prs = []
            for tk in range(NTS):
                ps_sc = psU.tile([P, S], F32, tag="u", name=f"sc{h}")
                for nch in range(NCH):
                    nc.tensor.matmul(
                        ps_sc[:, nch * CW:(nch + 1) * CW],
                        lhsT=qkh["k"][r0:r0 + HD, tk * P:(tk + 1) * P],
                        rhs=qkh["q"][r0:r0 + HD, nch * CW:(nch + 1) * CW],
                        start=True, stop=True)
                if tk >= 3:
                    at_matmuls(tk - 3, prs[tk - 3])
                pr = probs_pool.tile([P, S], BF16, tag="pr", name=f"pr{h}")
                nc.scalar.activation(out=pr, in_=ps_sc, func=AFT.Exp)
                # paced at 14 of 16 tk slots per head-pair so the prefetch
                # work stretches through hp=6 (instead of running dry early
                # and leaving the last head-pairs with no PE slack work);
                # qk(hp+2) still completes before heads(hp+2) start. Two of
                # the seven pieces go between the pipeline-drain at() calls
                # below, whose exps otherwise expose a sem-prop each.
                if piece_q and tk <= 4:
                    piece_q.pop(0)()
                prs.append(pr)
            for tt in (NTS - 3, NTS - 2):
                at_matmuls(tt, prs[tt])
                if piece_q:
                    piece_q.pop(0)()
            at_matmuls(NTS - 1, prs[NTS - 1])
            if last_hp:
                # evict straight from PSUM per 512-chunk: shortest exposed
                # latency before the out-proj chain's final accumulations.
                for nch in range(NCH):
                    rrow = rr_pool.tile([1, CW], F32, tag="rr", name=f"rr{h}{nch}")
                    nc.vector.reciprocal(out=rrow, in_=ps_at[nch][HD:HD + 1, :])
                    bc = bc_pool.tile([P, CW], F32, tag="bc", name=f"bcr{h}{nch}")
                    nc.gpsimd.partition_broadcast(out_ap=bc, in_ap=rrow)
                    nc.vector.tensor_tensor(
                        out=attnC[r0:r0 + HD, hp, nch * CW:(nch + 1) * CW],
                        in0=ps_at[nch][0:HD, :], in1=bc[0:HD, :], op=ALU.mult)
            else:
                for nch in range(NCH):
                    stg = stg_pool.tile([P, CW], F32, tag="stg",
                                        name=f"stg{h}{nch}")
                    nc.vector.tensor_copy(out=stg[0:HD + 1, :],
                                          in_=ps_at[nch][0:HD + 1, :])
                    rrow = rr_pool.tile([1, CW], F32, tag="rr",
                                        name=f"rr{h}{nch}")
                    nc.vector.reciprocal(out=rrow, in_=stg[HD:HD + 1, :])
                    bc = bc_pool.tile([P, CW], F32, tag="bc", name=f"bcr{h}{nch}")
                    nc.gpsimd.partition_broadcast(out_ap=bc, in_ap=rrow)
                    nc.vector.tensor_tensor(
                        out=attnC[r0:r0 + HD, hp, nch * CW:(nch + 1) * CW],
                        in0=stg[0:HD, :], in1=bc[0:HD, :], op=ALU.mult)
    fused.close()
    vaug_pool.release()
    if dbg:
        nc.sync.dma_start(out=dbg["attnC"],
                          in_=attnC.rearrange("p t s -> p (t s)"))

    # ---------------- out proj + residual + LN1 + transpose ----------------
    # x1 is kept in bf16 (it feeds fc1/the residual through bf16 matmuls
    # anyway); the d-major x1T copy comes from the DMA XBAR transpose, off
    # the PE entirely (was 64 PE transposes + 64 psum-evict copies).
    x1b16 = x1bp.tile([P, NTS, D], BF16, tag="x1b", name="x1b16")
    # two separate half-tiles (s 0:512 / 512:1024) so fc1's first chains
    # depend only on the first four transposes, not ts=7's late one
    # (dependencies are tile-granular for the DMA-transpose writes)
    x1T_a = big16.tile([P, NTD, CW], BF16, tag="big", name="x1T_a")
    x1T_h = [x1T_a, x1bp.tile([P, NTD, CW], BF16, tag="x1tb", name="x1T_b")]

    with tc.tile_pool(name="lnc", bufs=1) as ln_pool, \
         tc.tile_pool(name="tmpC", bufs=3) as tmpC:
        if not fast:
            g1_bc = bcast_tile(ln_pool, g1, queue=nc.sync)
            b1_bc = bcast_tile(ln_pool, b1, queue=nc.sync)

        def transpose_issue(tt):
            # last two transposes ride the scalar queue: the sync-queue DMA
            # counting semaphore guarding x1T_a then never includes them, so
            # fc1's first chains are not held hostage to ts=7's transpose
            eng = nc.scalar if tt >= NTS - 2 else nc.sync
            eng.dma_start(
                out=x1T_h[tt // 4][:, :, (tt % 4) * P:(tt % 4 + 1) * P],
                in_=x1b16[:, tt, :], transpose=True)

        for ts in range(NTS):
            ps = psU.tile([P, D], F32, tag="u", name="pso")
            for nch in range(NCH):
                for td in range(NTD):
                    nc.tensor.matmul(
                        ps[:, nch * CW:(nch + 1) * CW],
                        lhsT=attnC[:, td, ts * P:(ts + 1) * P],
                        rhs=wo_sb[:, td, nch * CW:(nch + 1) * CW],
                        start=(td == 0), stop=(td == NTD - 1))
            # LN1 with per-512-chunk residual add + stats: the first chunk's
            # work starts while the second chunk's matmul chain still runs
            x0 = tmpC.tile([P, D], F32, tag="x0", name="x0")
            st1 = tmpC.tile([P, 2, 6], F32, tag="lnst1", name="lnst1")
            for nch in range(NCH):
                cs = slice(nch * CW, (nch + 1) * CW)
                nc.vector.tensor_tensor(out=x0[:, cs], in0=ps[:, cs],
                                        in1=hs_sb[:, ts, cs], op=ALU.add)
            for nch in range(NCH):
                cs = slice(nch * CW, (nch + 1) * CW)
                nc.vector.bn_stats(out=st1[:, nch, :], in_=x0[:, cs])
            mv = tmpC.tile([P, 2], F32, tag="lnmv", name="lnmv")
            nc.vector.bn_aggr(out=mv, in_=st1)
            nc.scalar.activation(out=mv[:, 1:2], in_=mv[:, 1:2], func=AFT.Sqrt,
                                 bias=eps_t, scale=1.0)
            nc.vector.reciprocal(out=mv[:, 1:2], in_=mv[:, 1:2])
            bneg = tmpC.tile([P, 1], F32, tag="lnbneg", name="lnbneg")
            nc.vector.tensor_scalar(out=bneg, in0=mv[:, 0:1],
                                    scalar1=mv[:, 1:2], scalar2=-1.0,
                                    op0=ALU.mult, op1=ALU.mult)
            for nch in range(NCH):
                cs = slice(nch * CW, (nch + 1) * CW)
                nc.scalar.activation(out=x1b16[:, ts, cs], in_=x0[:, cs],
                                     func=AFT.Identity, bias=bneg,
                                     scale=mv[:, 1:2])
                if not fast:
                    nc.gpsimd.tensor_tensor(out=x1b16[:, ts, cs],
                                            in0=x1b16[:, ts, cs],
                                            in1=g1_bc[:, cs], op=ALU.mult)
                    badd = nc.gpsimd if ts % 2 == 0 else nc.vector
                    badd.tensor_tensor(out=x1b16[:, ts, cs],
                                       in0=x1b16[:, ts, cs],
                                       in1=b1_bc[:, cs], op=ALU.add)
            # transpose issued two iterations late: its xhat has completed,
            # so the dma_start never parks on the queue head waiting
            transpose_issue(ts)
        if dbg:
            nc.sync.dma_start(out=dbg["x1"],
                              in_=x1b16.rearrange("p t d -> p (t d)"))

    wo_pool.release()

    # ---------------- FFN (4 f-quarters, PSUM chains of 8) ----------------
    with tc.tile_pool(name="hTp", bufs=2) as hTp, \
         tc.tile_pool(name="fcb", bufs=1) as fcb_pool, \
         tc.tile_pool(name="tmpE", bufs=2) as tmpE:
        if not fast:
            f2b_bc = bcast_tile(fcb_pool, f2b, queue=nc.sync)
            g2_bc = bcast_tile(fcb_pool, g2, queue=nc.sync)
            b2_bc = bcast_tile(fcb_pool, b2, queue=nc.sync)
        out_r = out.rearrange("(t p) d -> p t d", p=P)
        out2 = None
        for q in range(NQ):
            w1, w2 = ffn_w0 if q == 0 else load_quarter(q)
            if q == 0:
                out2 = big32.tile([P, NTS, D], F32, tag="big", name="out2")
            hT_q = hTp.tile([P, FQT, S], BF16, tag="hT", name="hT_q")

            def fc1_chain(ft, nch, ps):
                for td in range(NTD):
                    nc.tensor.matmul(
                        ps[:, nch * CW:(nch + 1) * CW],
                        lhsT=w1[:, td, ft * P:(ft + 1) * P],
                        rhs=x1T_h[nch][:, td, :],
                        start=(td == 0), stop=(td == NTD - 1))

            def fc1_gelu(ft, ps):
                tf = q * FQT + ft
                if fast:
                    nc.scalar.activation(out=hT_q[:, ft, :], in_=ps,
                                         func=AFT.Gelu)
                else:
                    nc.scalar.activation(out=hT_q[:, ft, :], in_=ps,
                                         func=AFT.Gelu,
                                         bias=f1b_t[:, tf:tf + 1], scale=1.0)

            if q == 0:
                # nch1 chains trail three ft slots behind nch0: the second
                # x1T half (ts=7's transpose, finished only ~5us into the
                # FFN) is then never waited on by the PE.
                lag = 3
                ftps = {}
                for ft in range(FQT + lag):
                    if ft < FQT:
                        ftps[ft] = psU.tile([P, S], F32, tag="u", name="psh")
                        fc1_chain(ft, 0, ftps[ft])
                    if ft >= lag:
                        fc1_chain(ft - lag, 1, ftps[ft - lag])
                        fc1_gelu(ft - lag, ftps[ft - lag])
            else:
                for ft in range(FQT):
                    ps = psU.tile([P, S], F32, tag="u", name="psh")
                    for nch in range(NCH):
                        fc1_chain(ft, nch, ps)
                    fc1_gelu(ft, ps)
            if dbg and q == 0:
                nc.sync.dma_start(out=dbg["hT0"],
                                  in_=hT_q.rearrange("p t s -> p (t s)"))
            if dbg and q == NQ - 1:
                nc.sync.dma_start(out=dbg["out2"],
                                  in_=out2.rearrange("p t d -> p (t d)"))
            last_q = q == NQ - 1

            def q3_preload(ts):
                # preload out2 + x1 residual into PSUM; the final chain then
                # accumulates on top (start=False) and LN2 stats / xhat read
                # the finished PSUM directly — no SBUF evict-add is left on
                # the critical tail.
                pst = psU.tile([P, D], F32, tag="u", name="pso2")
                nc.vector.tensor_tensor(out=pst, in0=out2[:, ts, :],
                                        in1=x1b16[:, ts, :], op=ALU.add)
                return pst

            if last_q:
                ps_next = q3_preload(0)
            for ts in range(NTS):
                ps = ps_next if last_q else psU.tile([P, D], F32, tag="u",
                                                     name="pso2")
                for nch in range(NCH):
                    for ft in range(FQT):
                        nc.tensor.matmul(
                            ps[:, nch * CW:(nch + 1) * CW],
                            lhsT=hT_q[:, ft, ts * P:(ts + 1) * P],
                            rhs=w2[:, ft, nch * CW:(nch + 1) * CW],
                            start=(ft == 0 and not last_q),
                            stop=(ft == FQT - 1),
                            skip_group_check=last_q)
                if last_q and ts + 1 < NTS:
                    # next tile's preload issued BEFORE this tile's LN2: the
                    # 1127ns DVE preload otherwise queues behind the full LN2
                    # chain, delaying the final tile's stats at the tail
                    ps_next = q3_preload(ts + 1)
                if last_q:
                    # LN2 in quarter-chunks: the trailing chunk's stats/xhat/
                    # store are each ~350ns, minimizing the serial tail after
                    # the very last matmul.
                    NCK = 2
                    CKW = D // NCK
                    st2 = tmpE.tile([P, NCK, 6], F32, tag="lnst2", name="lnst2")
                    for ck in range(NCK):
                        cs = slice(ck * CKW, (ck + 1) * CKW)
                        nc.vector.bn_stats(out=st2[:, ck, :], in_=ps[:, cs])
                    mv = tmpE.tile([P, 2], F32, tag="lnmv2", name="lnmv2")
                    nc.vector.bn_aggr(out=mv, in_=st2)
                    nc.scalar.activation(out=mv[:, 1:2], in_=mv[:, 1:2],
                                         func=AFT.Sqrt, bias=eps_t, scale=1.0)
                    nc.vector.reciprocal(out=mv[:, 1:2], in_=mv[:, 1:2])
                    bneg = tmpE.tile([P, 1], F32, tag="lnbn2", name="lnbn2")
                    nc.vector.tensor_scalar(out=bneg, in0=mv[:, 0:1],
                                            scalar1=mv[:, 1:2], scalar2=-1.0,
                                            op0=ALU.mult, op1=ALU.mult)
                    yt = tmpE.tile([P, D], F32, tag="ye", name="ye")
                    for ck in range(NCK):
                        cs = slice(ck * CKW, (ck + 1) * CKW)
                        if fast and ts == NTS - 1 and ck == NCK - 1:
                            # last tile's second xhat half on DVE, in
                            # parallel with ACT's first half: xhat =
                            # ps*rstd + (-mean*rstd)
                            nc.vector.tensor_scalar(
                                out=yt[:, cs], in0=ps[:, cs],
                                scalar1=mv[:, 1:2], scalar2=bneg,
                                op0=ALU.mult, op1=ALU.add)
                        else:
                            nc.scalar.activation(out=yt[:, cs], in_=ps[:, cs],
                                                 func=AFT.Identity, bias=bneg,
                                                 scale=mv[:, 1:2])
                        if not fast:
                            nc.gpsimd.tensor_tensor(out=yt[:, cs], in0=yt[:, cs],
                                                    in1=g2_bc[:, cs], op=ALU.mult)
                            badd = nc.gpsimd if ts % 2 == 0 else nc.vector
                            badd.tensor_tensor(out=yt[:, cs], in0=yt[:, cs],
                                               in1=b2_bc[:, cs], op=ALU.add)
                        nc.sync.dma_start(out=out_r[:, ts, cs], in_=yt[:, cs])
                elif q == 0:
                    if fast:
                        nc.vector.tensor_copy(out=out2[:, ts, :], in_=ps)
                    else:
                        nc.vector.tensor_tensor(out=out2[:, ts, :], in0=ps,
                                                in1=f2b_bc, op=ALU.add)
                else:
                    nc.vector.tensor_tensor(out=out2[:, ts, :], in0=ps,
                                            in1=out2[:, ts, :], op=ALU.add)

    psQ.release()
    psU.release()
    x1bp.release()
    big32.release()
    big16.release()
    f1wp.release()
    f2wp.release()
    const.release()


# ---------------------------------------------------------------------------
# fp8 (e4m3) DoubleRow build: qkv / out-proj / fc1 matmuls at 0.5 cycles/row.
# Weights scaled x64 into fp8; descales folded into free spots:
#   exp(scale=2^-15) eats SCALING/64^2, gelu(scale=2^-6) eats fc1's x64,
#   v_aug ones-column = 8.0 makes the division emit attnC = 8*attn (fp8),
#   hs residual pre-scaled x512 on host (LN1 is scale-invariant).
# fc2 and the scores matmul stay bf16 (scores gain nothing from DR at
# contract=64; fc2-fp8 would push rel err past the 2e-2 gate).
# ---------------------------------------------------------------------------
FP8 = mybir.dt.float8e4
DRM = mybir.MatmulPerfMode.DoubleRow
SW = 64.0          # fp8 weight scale
EXPSC = 2.0 ** -15  # SCALING / SW^2
GELSC = 2.0 ** -6   # 1 / SW
NHP = 4            # NTD // 2: paired-d chain length
CK = 256           # DR moving chunk (rhs free = 2*CK = 512)


def build_fp8(dbg=False):
    nc = bacc.Bacc("TRN2", target_bir_lowering=False, debug=False)

    def din(name, shape, dt=F32):
        return nc.dram_tensor(name, shape, dt, kind="ExternalInput").ap()

    io = dict(
        hsT8=din("hsT8", (D, S), FP8),
        hs=din("hs", (S, D)),              # pre-scaled x512 on host
        wqb8=din("wqb8", (H // 2, P, D), FP8),   # [hp][p, (td, mt, 64)] x64
        wkb8=din("wkb8", (H // 2, P, D), FP8),
        wvT8=din("wvT8", (D, D), FP8),     # wv.T x64
        woT8=din("woT8", (D, D), FP8),     # wo.T x64
        f1w8=din("f1w8", (D, F), FP8),     # fc1_w.T x64
        f2w=din("f2w", (F, D), BF16),      # fc2_w.T
        out=nc.dram_tensor("out", (S, D), F32, kind="ExternalOutput").ap(),
        dbg=None,
    )
    if dbg:
        io["dbg"] = {
            "q0": nc.dram_tensor("dbg_q0", (P, S), BF16,
                                 kind="ExternalOutput").ap(),
            "k0": nc.dram_tensor("dbg_k0", (P, S), BF16,
                                 kind="ExternalOutput").ap(),
            "vaug": nc.dram_tensor("dbg_vaug", (P, NTS * H * (HD + 1)), BF16,
                                   kind="ExternalOutput").ap(),
            "pr0": nc.dram_tensor("dbg_pr0", (P, S), BF16,
                                  kind="ExternalOutput").ap(),
            "attnC": nc.dram_tensor("dbg_attnC", (P, NTD * S), FP8,
                                    kind="ExternalOutput").ap(),
            "x1": nc.dram_tensor("dbg_x1", (P, NTS * D), BF16,
                                 kind="ExternalOutput").ap(),
            "hT0": nc.dram_tensor("dbg_hT0", (P, FQT * S), BF16,
                                  kind="ExternalOutput").ap(),
            "out2": nc.dram_tensor("dbg_out2", (P, NTS * D), F32,
                                   kind="ExternalOutput").ap(),
        }
    with tile.TileContext(nc) as tc:
        _body_fp8(tc, io)
    nc.compile()
    return nc


def _body_fp8(tc, t):
    nc = tc.nc
    hsT8, hs = t["hsT8"], t["hs"]
    wqb8, wkb8, wvT8, woT8 = t["wqb8"], t["wkb8"], t["wvT8"], t["woT8"]
    f1w8, f2w = t["f1w8"], t["f2w"]
    out = t["out"]

    # ---- pools, LIFO release order ----
    const = tc.alloc_tile_pool(name="const", bufs=1)
    f2wp = tc.alloc_tile_pool(name="f2wp", bufs=2)
    f1wp = tc.alloc_tile_pool(name="f1wp", bufs=2)
    hsT8p = tc.alloc_tile_pool(name="hsT8p", bufs=1)
    big32 = tc.alloc_tile_pool(name="big32", bufs=1)
    x1bp = tc.alloc_tile_pool(name="x1bp", bufs=1)
    attnCp = tc.alloc_tile_pool(name="attnCp", bufs=1)
    psU = tc.alloc_tile_pool(name="psU", bufs=3, space="PSUM")
    psQ = tc.alloc_tile_pool(name="psQ", bufs=2, space="PSUM")
    wo_pool = tc.alloc_tile_pool(name="wo_pool", bufs=1)
    vaug_pool = tc.alloc_tile_pool(name="vaug_pool", bufs=1)

    eps_t = const.tile([P, 1], F32, tag="eps")
    nc.vector.memset(eps_t, EPS)
    ones_t = const.tile([P, 1], F32, tag="ones")
    nc.vector.memset(ones_t, 8.0)   # denominator slot scale -> attnC = 8*attn

    # ---- persistent tiles ----
    hsT_sb = hsT8p.tile([P, NTD, S], FP8, tag="hsT8", name="hsT_sb")
    hs_sb = big32.tile([P, NTS, D], F32, tag="big", name="hs_sb")
    v_aug = vaug_pool.tile([P, NTS, H, HD + 1], BF16, tag="vaug")
    wo_sb = wo_pool.tile([P, NTD, D], FP8, tag="wo")
    attnC = attnCp.tile([P, NTD, S], FP8, tag="attnC", name="attnC")

    hsT_r = hsT8.rearrange("(t p) s -> p t s", p=P)
    wvT_r = wvT8.rearrange("(t p) d -> p t d", p=P)
    f1w_r = f1w8.rearrange("(t p) f -> p t f", p=P)
    f2w_r = f2w.rearrange("(t p) d -> p t d", p=P)

    def load_quarter(q):
        w1 = f1wp.tile([P, NTD, FQ], FP8, tag="w1", name=f"w1q{q}")
        nc.sync.dma_start(out=w1, in_=f1w_r[:, :, q * FQ:(q + 1) * FQ])
        w2 = f2wp.tile([P, FQT, D], BF16, tag="w2", name=f"w2q{q}")
        nc.sync.dma_start(out=w2, in_=f2w_r[:, q * FQT:(q + 1) * FQT, :])
        return w1, w2

    # ones column of v_aug (denominator trick, x8)
    nc.vector.tensor_copy(
        out=v_aug[:, :, :, HD:HD + 1],
        in_=ones_t.unsqueeze(1).unsqueeze(1).broadcast_to((P, NTS, H, 1)))

    # ---------------- fused attention pools ----------------
    fused = ExitStack()
    qk_pool = fused.enter_context(tc.tile_pool(name="qkt", bufs=3))
    wqk_pool = fused.enter_context(tc.tile_pool(name="wqkp", bufs=2))
    probs_pool = fused.enter_context(tc.tile_pool(name="probs", bufs=5))
    bc_pool = fused.enter_context(tc.tile_pool(name="bcp", bufs=3))
    rr_pool = fused.enter_context(tc.tile_pool(name="rrp", bufs=4))
    stg_pool = fused.enter_context(tc.tile_pool(name="stgp", bufs=2))

    def qk_issue(hp):
        st = {"hp": hp}
        for wsrc, nm in ((wqb8, "q"), (wkb8, "k")):
            wblk = wqk_pool.tile([P, NTD, 2, HD], FP8, tag="w" + nm,
                                 name="w" + nm)
            nc.sync.dma_start(
                out=wblk,
                in_=wsrc[hp].rearrange("p (t m c) -> p t m c", t=NTD, m=2))
            st["w" + nm] = wblk
            st[nm] = qk_pool.tile([P, S], BF16, tag=nm + "T", name=nm + "Th")
        return st

    def qk_pieces(st):
        """DR qk chains for one head-pair as 16 four-matmul thunks.
        Chain (nm, mt, sh): psum [64, 512], two 256-chunk sub-chains of 4
        DR matmuls; evict to the bf16 q/k tile after the second chunk."""
        thunks = []
        for nm in ("q", "k"):
            for mt in range(2):
                for sh in range(2):
                    holder = {}
                    for ck in range(2):
                        def piece(nm=nm, mt=mt, sh=sh, ck=ck, holder=holder,
                                  st=st):
                            if ck == 0:
                                holder["ps"] = psQ.tile([HD, CW], F32,
                                                        tag="q5", name="psq")
                            ps = holder["ps"]
                            s0 = sh * CW + ck * CK
                            for tdp in range(NHP):
                                nc.tensor.matmul(
                                    ps[:, ck * CK:(ck + 1) * CK],
                                    lhsT=st["w" + nm][:, 2 * tdp:2 * tdp + 2,
                                                      mt, :],
                                    rhs=hsT_sb[:, 2 * tdp:2 * tdp + 2,
                                               s0:s0 + CK],
                                    start=(tdp == 0), stop=(tdp == NHP - 1),
                                    perf_mode=DRM)
                            if ck == 1:
                                nc.vector.tensor_copy(
                                    out=st[nm][mt * HD:(mt + 1) * HD,
                                               sh * CW:(sh + 1) * CW],
                                    in_=ps)
                        thunks.append(piece)
        return thunks

    def qk_compute(st):
        for th in qk_pieces(st):
            th()
        return st

    # lead-in: wq0 -> hsT8 -> wk0; qk chains stream behind the slice DMAs
    st0 = {"hp": 0}
    st0["wq"] = wqk_pool.tile([P, NTD, 2, HD], FP8, tag="wq", name="wq")
    nc.sync.dma_start(
        out=st0["wq"],
        in_=wqb8[0].rearrange("p (t m c) -> p t m c", t=NTD, m=2))
    for i in range(0, 2):
        nc.sync.dma_start(out=hsT_sb[:, i, :], in_=hsT_r[:, i, :])
    st0["wk"] = wqk_pool.tile([P, NTD, 2, HD], FP8, tag="wk", name="wk")
    nc.scalar.dma_start(
        out=st0["wk"],
        in_=wkb8[0].rearrange("p (t m c) -> p t m c", t=NTD, m=2))
    for i in range(2, NTD):
        # startup feed: alternate queues so the slice DMAs overlap
        eng = (nc.sync, nc.scalar, nc.gpsimd)[i % 3]
        eng.dma_start(out=hsT_sb[:, i, :], in_=hsT_r[:, i, :])
    st0["q"] = qk_pool.tile([P, S], BF16, tag="qT", name="qTh")
    st0["k"] = qk_pool.tile([P, S], BF16, tag="kT", name="kTh")
    qk_pipe = [qk_compute(st0), qk_compute(qk_issue(1))]
    dbg = t.get("dbg")
    if dbg:
        nc.sync.dma_start(out=dbg["q0"], in_=st0["q"])
        nc.sync.dma_start(out=dbg["k0"], in_=st0["k"])

    # wv8 rides the w1 slot (same shape/dtype); released to w1q0 after v-proj
    wv_sb = f1wp.tile([P, NTD, FQ], FP8, tag="w1", name="wv_sb")
    qt = NTD // 4
    for i in range(4):
        nc.sync.dma_start(out=wv_sb[:, i * qt:(i + 1) * qt, :],
                          in_=wvT_r[:, i * qt:(i + 1) * qt, :])

    # ---------------- v projection (DR: hsT stationary, wv moving) --------
    for ts in range(NTS):
        for mt in range(2):
            ps = psU.tile([HD, D], F32, tag="u", name="psv")
            for ck in range(4):
                for tdp in range(NHP):
                    nc.tensor.matmul(
                        ps[:, ck * CK:(ck + 1) * CK],
                        lhsT=hsT_sb[:, 2 * tdp:2 * tdp + 2,
                                    ts * P + mt * HD:ts * P + (mt + 1) * HD],
                        rhs=wv_sb[:, 2 * tdp:2 * tdp + 2, ck * CK:(ck + 1) * CK],
                        start=(tdp == 0), stop=(tdp == NHP - 1),
                        perf_mode=DRM)
            nc.vector.tensor_copy(
                out=v_aug[mt * HD:(mt + 1) * HD, ts, :, 0:HD],
                in_=ps.rearrange("p (h e) -> p h e", h=H))

    if dbg:
        nc.sync.dma_start(
            out=dbg["vaug"],
            in_=v_aug.rearrange("p t h e -> p (t h e)"))

    # ---------------- attention (qk prefetched two head-pairs ahead) ------
    hs_r = hs.rearrange("(t p) d -> p t d", p=P)
    ffn_w0 = None

    piece_q = []
    for hp in range(H // 2):
        qkh = qk_pipe.pop(0)
        if hp + 2 < H // 2:
            st_next = qk_issue(hp + 2)
            qk_pipe.append(st_next)
            piece_q.extend(qk_pieces(st_next))
        if hp == 0:
            nc.sync.dma_start(out=hs_sb, in_=hs_r)
        if hp == 1:
            nc.sync.dma_start(
                out=wo_sb, in_=woT8.rearrange("(t p) d -> p t d", p=P))
            ffn_w0 = load_quarter(0)
        for h in (2 * hp, 2 * hp + 1):
            r0 = (h % 2) * HD
            last_hp = (hp == H // 2 - 1)
            if last_hp:
                ps_at = [psQ.tile([P, CW], F32, tag="q5", name=f"atq{h}{c}")
                         for c in range(NCH)]
            else:
                at_full = psU.tile([P, S], F32, tag="u", name=f"at{h}")
                ps_at = [at_full[:, c * CW:(c + 1) * CW] for c in range(NCH)]

            def at_matmuls(tk, pr):
                for nch in range(NCH):
                    nc.tensor.matmul(
                        ps_at[nch][0:HD + 1, :],
                        lhsT=v_aug[:, tk, h, :],
                        rhs=pr[:, nch * CW:(nch + 1) * CW],
                        start=(tk == 0), stop=(tk == NTS - 1))

            prs = []
            for tk in range(NTS):
                ps_sc = psU.tile([P, S], F32, tag="u", name=f"sc{h}")
                for nch in range(NCH):
                    nc.tensor.matmul(
                        ps_sc[:, nch * CW:(nch + 1) * CW],
                        lhsT=qkh["k"][r0:r0 + HD, tk * P:(tk + 1) * P],
                        rhs=qkh["q"][r0:r0 + HD, nch * CW:(nch + 1) * CW],
                        start=True, stop=True)
                if tk >= 3:
                    at_matmuls(tk - 3, prs[tk - 3])
                pr = probs_pool.tile([P, S], BF16, tag="pr", name=f"pr{h}")
                nc.scalar.activation(out=pr, in_=ps_sc, func=AFT.Exp,
                                     scale=EXPSC)
                if dbg and h == 0 and tk == 0:
                    nc.sync.dma_start(out=dbg["pr0"], in_=pr)
                if piece_q and tk <= 5:
                    piece_q.pop(0)()
                prs.append(pr)
            for tt in (NTS - 3, NTS - 2):
                at_matmuls(tt, prs[tt])
                if piece_q:
                    piece_q.pop(0)()
            at_matmuls(NTS - 1, prs[NTS - 1])
            if last_hp:
                for nch in range(NCH):
                    rrow = rr_pool.tile([1, CW], F32, tag="rr", name=f"rr{h}{nch}")
                    nc.vector.reciprocal(out=rrow, in_=ps_at[nch][HD:HD + 1, :])
                    bc = bc_pool.tile([P, CW], F32, tag="bc", name=f"bcr{h}{nch}")
                    nc.gpsimd.partition_broadcast(out_ap=bc, in_ap=rrow)
                    nc.vector.tensor_tensor(
                        out=attnC[r0:r0 + HD, hp, nch * CW:(nch + 1) * CW],
                        in0=ps_at[nch][0:HD, :], in1=bc[0:HD, :], op=ALU.mult)
            else:
                for nch in range(NCH):
                    stg = stg_pool.tile([P, CW], F32, tag="stg",
                                        name=f"stg{h}{nch}")
                    nc.vector.tensor_copy(out=stg[0:HD + 1, :],
                                          in_=ps_at[nch][0:HD + 1, :])
                    rrow = rr_pool.tile([1, CW], F32, tag="rr",
                                        name=f"rr{h}{nch}")
                    nc.vector.reciprocal(out=rrow, in_=stg[HD:HD + 1, :])
                    bc = bc_pool.tile([P, CW], F32, tag="bc", name=f"bcr{h}{nch}")
                    nc.gpsimd.partition_broadcast(out_ap=bc, in_ap=rrow)
                    nc.vector.tensor_tensor(
                        out=attnC[r0:r0 + HD, hp, nch * CW:(nch + 1) * CW],
                        in0=stg[0:HD, :], in1=bc[0:HD, :], op=ALU.mult)
    fused.close()
    vaug_pool.release()
    if dbg:
        nc.sync.dma_start(out=dbg["attnC"],
                          in_=attnC.rearrange("p t s -> p (t s)"))

    # ---------------- out proj (DR) + residual + LN1 + transpose ----------
    x1b16 = x1bp.tile([P, NTS, D], BF16, tag="x1b", name="x1b16")
    x1T_h = [x1bp.tile([P, NTD, CW], BF16, tag="x1ta", name="x1T_a"),
             x1bp.tile([P, NTD, CW], BF16, tag="x1tb", name="x1T_b")]
    x1T8 = x1bp.tile([P, NTD, S], FP8, tag="x1t8", name="x1T8")

    with tc.tile_pool(name="tmpC", bufs=3) as tmpC:

        def transpose_issue(tt):
            # scalar (ACT) queue: parking there is harmless during out-proj,
            # and the sync queue keeps feeding weight/output DMAs
            nc.scalar.dma_start(
                out=x1T_h[tt // 4][:, :, (tt % 4) * P:(tt % 4 + 1) * P],
                in_=x1b16[:, tt, :], transpose=True)
            # bf16 -> fp8 convert for the DR fc1 operand (sbuf->sbuf); the
            # last two go to DVE so they don't serialize on the Pool FIFO
            # behind ts 4/5 right when fc1's tail chunks need them
            eng = nc.vector if tt >= NTS - 2 else nc.gpsimd
            eng.tensor_copy(
                out=x1T8[:, :, tt * P:(tt + 1) * P],
                in_=x1T_h[tt // 4][:, :, (tt % 4) * P:(tt % 4 + 1) * P])

        for ts in range(NTS):
            ps_mt = []
            for mt in range(2):
                ps = psU.tile([HD, D], F32, tag="u", name="pso")
                for ck in range(4):
                    for tdp in range(NHP):
                        nc.tensor.matmul(
                            ps[:, ck * CK:(ck + 1) * CK],
                            lhsT=attnC[:, 2 * tdp:2 * tdp + 2,
                                       ts * P + mt * HD:ts * P + (mt + 1) * HD],
                            rhs=wo_sb[:, 2 * tdp:2 * tdp + 2,
                                      ck * CK:(ck + 1) * CK],
                            start=(tdp == 0), stop=(tdp == NHP - 1),
                            perf_mode=DRM)
                ps_mt.append(ps)
            # residual add (hs pre-scaled x512 host-side; LN1 invariant).
            # PSUM in0 + SB in1 at different partition bases is legal (the
            # same-base rule only applies when both inputs are in SB).
            x0 = tmpC.tile([P, D], F32, tag="x0", name="x0")
            st1 = tmpC.tile([P, 2, 6], F32, tag="lnst1", name="lnst1")
            for mt in range(2):
                nc.vector.tensor_tensor(
                    out=x0[mt * HD:(mt + 1) * HD, :], in0=ps_mt[mt],
                    in1=hs_sb[mt * HD:(mt + 1) * HD, ts, :], op=ALU.add)
            for nch in range(NCH):
                cs = slice(nch * CW, (nch + 1) * CW)
                nc.vector.bn_stats(out=st1[:, nch, :], in_=x0[:, cs])
            mv = tmpC.tile([P, 2], F32, tag="lnmv", name="lnmv")
            nc.vector.bn_aggr(out=mv, in_=st1)
            nc.scalar.activation(out=mv[:, 1:2], in_=mv[:, 1:2], func=AFT.Sqrt,
                                 bias=eps_t, scale=1.0)
            nc.vector.reciprocal(out=mv[:, 1:2], in_=mv[:, 1:2])
            bneg = tmpC.tile([P, 1], F32, tag="lnbneg", name="lnbneg")
            nc.vector.tensor_scalar(out=bneg, in0=mv[:, 0:1],
                                    scalar1=mv[:, 1:2], scalar2=-1.0,
                                    op0=ALU.mult, op1=ALU.mult)
            for nch in range(NCH):
                cs = slice(nch * CW, (nch + 1) * CW)
                nc.scalar.activation(out=x1b16[:, ts, cs], in_=x0[:, cs],
                                     func=AFT.Identity, bias=bneg,
                                     scale=mv[:, 1:2])
            transpose_issue(ts)
        if dbg:
            nc.sync.dma_start(out=dbg["x1"],
                              in_=x1b16.rearrange("p t d -> p (t d)"))

    wo_pool.release()
    attnCp.release()

    # ---------------- FFN (fc1 fp8-DR, fc2 bf16; 4 f-quarters) ------------
    # fc2(q-1) chains interleave into fc1(q)'s stretch: the gelu evictions
    # (1.03us per [64,S] on ACT) otherwise rate-limit the 0.85us fc1 chains
    # through the 3-buf psum rotation.
    with tc.tile_pool(name="hTp", bufs=2) as hTp, \
         tc.tile_pool(name="tmpE", bufs=2) as tmpE:
        out_r = out.rearrange("(t p) d -> p t d", p=P)
        out2 = big32.tile([P, NTS, D], F32, tag="big", name="out2")

        def fc1_unit(w1, hT_q, ft64):
            ps = psU.tile([HD, S], F32, tag="u", name="psh")
            f0 = ft64 * HD
            for ck in range(4):
                for tdp in range(NHP):
                    nc.tensor.matmul(
                        ps[:, ck * CK:(ck + 1) * CK],
                        lhsT=w1[:, 2 * tdp:2 * tdp + 2, f0:f0 + HD],
                        rhs=x1T8[:, 2 * tdp:2 * tdp + 2,
                                 ck * CK:(ck + 1) * CK],
                        start=(tdp == 0), stop=(tdp == NHP - 1),
                        perf_mode=DRM)
            nc.scalar.activation(
                out=hT_q[(ft64 % 2) * HD:(ft64 % 2 + 1) * HD, ft64 // 2, :],
                in_=ps, func=AFT.Gelu, scale=GELSC)

        def fc2_unit(hT_p, w2_p, qq, ts):
            ps = psU.tile([P, D], F32, tag="u", name="pso2")
            for nch in range(NCH):
                for ft in range(FQT):
                    nc.tensor.matmul(
                        ps[:, nch * CW:(nch + 1) * CW],
                        lhsT=hT_p[:, ft, ts * P:(ts + 1) * P],
                        rhs=w2_p[:, ft, nch * CW:(nch + 1) * CW],
                        start=(ft == 0), stop=(ft == FQT - 1))
            if qq == 0:
                nc.vector.tensor_copy(out=out2[:, ts, :], in_=ps)
            else:
                nc.vector.tensor_tensor(out=out2[:, ts, :], in0=ps,
                                        in1=out2[:, ts, :], op=ALU.add)
            return ps

        prev = None
        wq_next = ffn_w0
        for q in range(NQ):
            w1, w2 = wq_next
            hT_q = hTp.tile([P, FQT, S], BF16, tag="hT", name="hT_q")
            for ft64 in range(16):
                fc1_unit(w1, hT_q, ft64)
                if ft64 == 1 and q + 1 < NQ:
                    # prefetch next quarter's weights into the second buffer
                    wq_next = load_quarter(q + 1)
                if prev is not None and ft64 % 2 == 1:
                    fc2_unit(*prev, ft64 // 2)
            if dbg and q == 0:
                nc.sync.dma_start(out=dbg["hT0"],
                                  in_=hT_q.rearrange("p t s -> p (t s)"))
            prev = (hT_q, w2, q)

        if dbg:
            nc.sync.dma_start(out=dbg["out2"],
                              in_=out2.rearrange("p t d -> p (t d)"))
        # pre-merge out2 += x1 on Pool (off the DVE critical path; a DVE
        # preload into PSUM misbehaves on banks whose last matmul group was
        # fp8-DR, so LN2 reads a plain SBUF sum instead)
        for ts in range(NTS):
            nc.gpsimd.tensor_tensor(out=out2[:, ts, :], in0=out2[:, ts, :],
                                    in1=x1b16[:, ts, :], op=ALU.add)
        hT_p, w2_p, _ = prev
        for ts in range(NTS):
            ps = psU.tile([P, D], F32, tag="u", name="pso3")
            for nch in range(NCH):
                for ft in range(FQT):
                    nc.tensor.matmul(
                        ps[:, nch * CW:(nch + 1) * CW],
                        lhsT=hT_p[:, ft, ts * P:(ts + 1) * P],
                        rhs=w2_p[:, ft, nch * CW:(nch + 1) * CW],
                        start=(ft == 0), stop=(ft == FQT - 1))
            NCK = 4 if ts == NTS - 1 else 2
            CKW = D // NCK
            x2 = tmpE.tile([P, D], F32, tag="x2", name="x2")
            st2 = tmpE.tile([P, 4, 6], F32, tag="lnst2", name="lnst2")
            for ck in range(NCK):
                cs = slice(ck * CKW, (ck + 1) * CKW)
                nc.vector.tensor_tensor(out=x2[:, cs], in0=ps[:, cs],
                                        in1=out2[:, ts, cs], op=ALU.add)
                nc.vector.bn_stats(out=st2[:, ck, :], in_=x2[:, cs])
            mv = tmpE.tile([P, 2], F32, tag="lnmv2", name="lnmv2")
            nc.vector.bn_aggr(out=mv, in_=st2[:, 0:NCK, :])
            nc.scalar.activation(out=mv[:, 1:2], in_=mv[:, 1:2],
                                 func=AFT.Sqrt, bias=eps_t, scale=1.0)
            nc.vector.reciprocal(out=mv[:, 1:2], in_=mv[:, 1:2])
            bneg = tmpE.tile([P, 1], F32, tag="lnbn2", name="lnbn2")
            nc.vector.tensor_scalar(out=bneg, in0=mv[:, 0:1],
                                    scalar1=mv[:, 1:2], scalar2=-1.0,
                                    op0=ALU.mult, op1=ALU.mult)
            yt = tmpE.tile([P, D], F32, tag="ye", name="ye")
            for ck in range(NCK):
                cs = slice(ck * CKW, (ck + 1) * CKW)
                if ts == NTS - 1 and ck == NCK - 1:
                    nc.vector.tensor_scalar(
                        out=yt[:, cs], in0=x2[:, cs],
                        scalar1=mv[:, 1:2], scalar2=bneg,
                        op0=ALU.mult, op1=ALU.add)
                else:
                    nc.scalar.activation(out=yt[:, cs], in_=x2[:, cs],
                                         func=AFT.Identity, bias=bneg,
                                         scale=mv[:, 1:2])
                nc.sync.dma_start(out=out_r[:, ts, cs], in_=yt[:, cs])

    psQ.release()
    psU.release()
    x1bp.release()
    big32.release()
    hsT8p.release()
    f1wp.release()
    f2wp.release()
    const.release()


# ---------------------------------------------------------------------------
# Full-input entry point: data-parallel over batch across 8 NeuronCores.
# ---------------------------------------------------------------------------
import numpy as np
import ml_dtypes
from concourse import bass_utils

B = 8
SCALING = HD ** -0.5
BF = ml_dtypes.bfloat16
E4M3 = ml_dtypes.float8_e4m3

_NC_CACHE = {}


def _get_nc(fast=True):
    if fast not in _NC_CACHE:
        _NC_CACHE[fast] = build_fp8() if fast else build(fast=False)
    return _NC_CACHE[fast]


def _blk(wt):
    """[D, D] -> (H/2, P, D) with wblk[hp][p, ti*P+c] = wt[ti*P+p, hp*P+c]."""
    a = np.asarray(wt, dtype=np.float32).reshape(D // P, P, H // 2, P)
    return np.ascontiguousarray(
        a.transpose(2, 1, 0, 3).reshape(H // 2, P, D)).astype(BF)


def _prep_core_inputs(b_hs, w):
    c = np.ascontiguousarray
    f = np.float32

    def a(x):
        return c(np.asarray(x)).astype(f, copy=False)

    def ab(x):
        return c(np.asarray(x, dtype=f)).astype(BF)

    return {
        "hsT": ab(np.asarray(b_hs).T),
        "hs": a(b_hs),
        "wqb": _blk(np.asarray(w["wq"], dtype=f).T * SCALING),
        "wkb": _blk(np.asarray(w["wk"], dtype=f).T),
        "wvT": ab(np.asarray(w["wv"], dtype=f).T),
        "woT": ab(np.asarray(w["wo"], dtype=f).T),
        "bq": a(np.asarray(w["bq"], dtype=f) * SCALING),
        "bk": a(w["bk"]),
        "bv": a(w["bv"]),
        "bo": a(w["bo"]),
        "g1": a(w["ln1_g"]),
        "b1": a(w["ln1_b"]),
        "g2": a(w["ln2_g"]),
        "b2": a(w["ln2_b"]),
        "f1w": ab(np.asarray(w["fc1_w"], dtype=f).T),
        "f1b": a(w["fc1_b"]),
        "f2w": ab(np.asarray(w["fc2_w"], dtype=f).T),
        "f2b": a(w["fc2_b"]),
    }


def _blk8(wt):
    """wt [D, D] (dout, din) -> fp8 [H/2, P, D] with
    blk[hp][p, td*128 + mt*64 + c] = wt.T[128*td + p, 128*hp + 64*mt + c]."""
    a = (np.asarray(wt, np.float32).T * SW).astype(E4M3)
    a = a.reshape(NTD, P, H // 2, 2, HD)        # [td, p, hp, mt, c]
    return np.ascontiguousarray(
        a.transpose(2, 1, 0, 3, 4).reshape(H // 2, P, D))


def _prep_core_inputs_fp8(b_hs, w):
    c = np.ascontiguousarray
    f = np.float32

    def a8(x, scale=SW):
        return c(np.asarray(x, dtype=f).T * scale).astype(E4M3)

    hs32 = np.asarray(b_hs, dtype=f)
    return {
        "hsT8": c(hs32.T).astype(E4M3),
        "hs": c(hs32 * 512.0),           # attnC(8) * wo(64); LN1 invariant
        "wqb8": _blk8(w["wq"]),
        "wkb8": _blk8(w["wk"]),
        "wvT8": a8(w["wv"]),
        "woT8": a8(w["wo"]),
        "f1w8": a8(w["fc1_w"]),
        "f2w": c(np.asarray(w["fc2_w"], dtype=f).T).astype(BF),
    }


def kernel(**inputs):
    """Takes full unsharded inputs (setup_inputs() keys), returns [B, S, D]."""
    w = {k: np.asarray(v) for k, v in inputs.items()}
    hs_all = w["hidden_states"]
    assert hs_all.shape == (B, S, D), hs_all.shape
    fast = all(
        np.all(np.asarray(w[k]) == 0.0)
        for k in ("bq", "bk", "bv", "bo", "fc1_b", "fc2_b", "ln1_b", "ln2_b")
    ) and all(np.all(np.asarray(w[k]) == 1.0) for k in ("ln1_g", "ln2_g"))
    nc = _get_nc(fast)
    prep = _prep_core_inputs_fp8 if fast else _prep_core_inputs
    in_maps = [prep(hs_all[c], w) for c in range(B)]
    res = bass_utils.run_bass_kernel_spmd(nc, in_maps, core_ids=list(range(B)))
    out_full = np.stack([res.results[c]["out"] for c in range(B)])
    return out_full.astype(np.float32, copy=False)

